# revision 1
# baseline (speedup 1.0000x reference)
"""Transformer block (pre-norm attention + MLP) on 8 TRN2 NeuronCores.

Sharding: 8 cores = 4 batch elements x 2 sequence halves (data parallel, no
collectives). Each core computes its 1024 "own" query tokens end-to-end and
redundantly builds K/V for the full 2048-token batch element. The k-token
order is permuted per core (own tokens first) so the SPMD program is
identical on every core — softmax over k is permutation invariant.

All matmuls run with bf16 operands (fp32 PSUM accumulation); the residual
path stays fp32. LayerNorm affine params are folded into the adjacent matmul
weights host-side. Softmax skips max-subtraction (|scores| <= ~10 here) and
gets its denominators for free from an appended ones-column on V.
"""

from contextlib import ExitStack

import numpy as np

try:
    import jax
    jax.config.update("jax_compilation_cache_dir", "/tmp/jax_bass_cache")
    jax.config.update("jax_persistent_cache_min_compile_time_secs", 0.0)
    jax.config.update("jax_persistent_cache_min_entry_size_bytes", -1)
except Exception:
    import jax

import concourse.bacc as bacc
import concourse.bass as bass
import concourse.mybir as mybir
import concourse.tile as tile
from concourse.masks import make_identity

FP32 = mybir.dt.float32
BF16 = mybir.dt.bfloat16
AF = mybir.ActivationFunctionType
ALU = mybir.AluOpType

D = 1024          # model dim
DT = 8            # d tiles of 128
H = 16            # heads
HD = 64           # head dim
HID = 4096        # mlp hidden
T_ALL = 2048      # tokens per core incl. K/V-only tokens
T_OWN = 1024      # query/output tokens per core
EPS = 1e-6
N_CORES = 8


def _ln_transpose(nc, statp, znp, trp, src_getter, n_tiles, zt_out, eps_sb, ident):
    """LayerNorm (w/b folded into the following matmul weights host-side)
    + PE transpose into zt_out [128, DT, n_tiles*128] bf16."""
    for tt in range(n_tiles):
        xt = src_getter(tt)  # [128, D] fp32 sbuf tile
        stats = statp.tile([128, 2, 6], FP32, tag="stats")
        nc.vector.bn_stats(out=stats[:, 0, :], in_=xt[:, 0:512])
        nc.vector.bn_stats(out=stats[:, 1, :], in_=xt[:, 512:1024])
        mv = statp.tile([128, 2], FP32, tag="mv")
        nc.vector.bn_aggr(out=mv, in_=stats)
        sd = statp.tile([128, 1], FP32, tag="sd")
        nc.scalar.activation(out=sd, in_=mv[:, 1:2], func=AF.Sqrt, bias=eps_sb)
        rinv = statp.tile([128, 1], FP32, tag="rinv")
        nc.vector.reciprocal(out=rinv, in_=sd)
        zn = znp.tile([128, D], BF16, tag="zn")
        nc.vector.tensor_scalar(
            out=zn, in0=xt, scalar1=mv[:, 0:1], scalar2=rinv,
            op0=ALU.subtract, op1=ALU.mult,
        )
        for g in range(2):
            ps = trp.tile([128, 4, 128], BF16, tag="trps")
            for i in range(4):
                nc.tensor.transpose(
                    ps[:, i, :], zn[:, (4 * g + i) * 128:(4 * g + i + 1) * 128], ident
                )
            nc.vector.tensor_copy(
                out=zt_out[:, 4 * g:4 * g + 4, tt * 128:(tt + 1) * 128], in_=ps
            )


def build_nc():
    nc = bacc.Bacc("TRN2", target_bir_lowering=False, debug=False,
                   num_devices=N_CORES)

    x = nc.dram_tensor("x", [T_ALL, D], FP32, kind="ExternalInput")
    wqkv = nc.dram_tensor("wqkv", [D, 3 * D], BF16, kind="ExternalInput")
    bqkv = nc.dram_tensor("bqkv", [3 * D], FP32, kind="ExternalInput")
    wproj = nc.dram_tensor("wproj", [D, D], BF16, kind="ExternalInput")
    bproj = nc.dram_tensor("bproj", [D], BF16, kind="ExternalInput")
    w1 = nc.dram_tensor("w1", [D, HID], BF16, kind="ExternalInput")
    b1 = nc.dram_tensor("b1", [HID], FP32, kind="ExternalInput")
    w2 = nc.dram_tensor("w2", [HID, D], BF16, kind="ExternalInput")
    b2 = nc.dram_tensor("b2", [D], BF16, kind="ExternalInput")
    y = nc.dram_tensor("y", [T_OWN, D], FP32, kind="ExternalOutput")

    wqkv_t = wqkv.ap().rearrange("(dt p) f -> p dt f", p=128)   # [128, 8, 3072]
    w1_t = w1.ap().rearrange("(dt p) f -> p dt f", p=128)       # [128, 8, 4096]
    w2_t = w2.ap().rearrange("(jt p) f -> p jt f", p=128)       # [128, 32, 1024]

    with tile.TileContext(nc) as tc, ExitStack() as ctx:
        P = ctx.enter_context

        # ---- whole-kernel pools ----
        singles = P(tc.tile_pool(name="singles", bufs=1))
        xpool = P(tc.tile_pool(name="xin", bufs=3))
        statpool = P(tc.tile_pool(name="stat", bufs=4))
        znpool = P(tc.tile_pool(name="zn", bufs=3))
        ypool = P(tc.tile_pool(name="yp", bufs=3))

        # ---- constants ----
        ident = singles.tile([128, 128], BF16)
        make_identity(nc, ident)
        ones_bf = singles.tile([1, 128], BF16)
        nc.vector.memset(ones_bf, 1.0)
        eps_sb = singles.tile([128, 1], FP32)
        nc.vector.memset(eps_sb, EPS)
        bq_sb = singles.tile([128, 24], FP32)
        nc.sync.dma_start(out=bq_sb, in_=bqkv.ap().rearrange("(f p) -> p f", p=128))
        b1_sb = singles.tile([128, 32], FP32)
        nc.sync.dma_start(out=b1_sb, in_=b1.ap().rearrange("(f p) -> p f", p=128))
        bproj_sb = singles.tile([1, D], BF16)
        nc.sync.dma_start(out=bproj_sb, in_=bproj.ap().rearrange("(o f) -> o f", o=1))
        b2_sb = singles.tile([1, D], BF16)
        nc.sync.dma_start(out=b2_sb, in_=b2.ap().rearrange("(o f) -> o f", o=1))
        # V-bias broadcast to all partitions [128, 1024]
        vbias_sb = singles.tile([128, D], FP32)
        nc.sync.dma_start(
            out=vbias_sb,
            in_=bass.AP(tensor=bqkv, offset=2 * D, ap=[[0, 128], [1, D]]),
        )

        # ---- phase A: LN1 + transpose -> z1T ----
        es_z1 = ExitStack()
        z1p = es_z1.enter_context(tc.tile_pool(name="z1p", bufs=1, side="right"))
        z1T = z1p.tile([128, DT, T_ALL], BF16, tag="z1T")

        def load_x(tt):
            xt = xpool.tile([128, D], FP32, tag="xa")
            nc.sync.dma_start(out=xt, in_=x[tt * 128:(tt + 1) * 128, :])
            return xt

        with tc.tile_pool(name="psA", bufs=2, space="PSUM") as trpsA:
            _ln_transpose(nc, statpool, znpool, trpsA, load_x, T_ALL // 128,
                          z1T, eps_sb, ident)

        # ---- phase B: QKV ----
        es_kqv = ExitStack()
        kqvp = es_kqv.enter_context(tc.tile_pool(name="kqvp", bufs=1))
        kt_all = kqvp.tile([128, DT, T_ALL], BF16, tag="kt")
        qt_all = kqvp.tile([128, DT, T_OWN], BF16, tag="qt")
        VP = kqvp.tile([128, 16, 16 * (HD + 1)], BF16, tag="vp")
        # ones columns of V'
        vp_ones = VP.rearrange("p k (h e) -> p k h e", e=HD + 1)[:, :, :, HD:HD + 1]
        nc.vector.memset(vp_ones, 1.0)

        with (
            tc.tile_pool(name="wq", bufs=2, side="right") as wq_pool,
            tc.tile_pool(name="psB", bufs=2, space="PSUM") as qkpsum,
        ):
            # K and Q feature tiles, interleaved so attention unblocks early
            forder = []
            for i in range(8):
                forder += [8 + i, i]
            for f in forder:
                wq_f = wq_pool.tile([128, DT, 128], BF16, tag="wqf")
                nc.sync.dma_start(out=wq_f, in_=wqkv_t[:, :, f * 128:(f + 1) * 128])
                nch = 4 if f >= 8 else 2
                for tcn in range(nch):
                    ps = qkpsum.tile([128, 512], FP32, tag="qkps")
                    for d in range(DT):
                        nc.tensor.matmul(
                            ps, wq_f[:, d, :], z1T[:, d, tcn * 512:(tcn + 1) * 512],
                            start=(d == 0), stop=(d == DT - 1),
                        )
                    if f >= 8:
                        dst = kt_all[:, f - 8, tcn * 512:(tcn + 1) * 512]
                    else:
                        dst = qt_all[:, f, tcn * 512:(tcn + 1) * 512]
                    nc.vector.tensor_scalar(
                        out=dst, in0=ps, scalar1=bq_sb[:, f:f + 1], scalar2=None,
                        op0=ALU.add,
                    )

            # V in natural layout into V' (with bias, strided 65-col head groups)
            for vc in range(2):
                wv = wq_pool.tile([128, DT, 512], BF16, tag="wvf")
                nc.sync.dma_start(
                    out=wv, in_=wqkv_t[:, :, 2 * D + vc * 512:2 * D + (vc + 1) * 512]
                )
                for tt in range(T_ALL // 128):
                    ps = qkpsum.tile([128, 512], FP32, tag="qkps")
                    for d in range(DT):
                        nc.tensor.matmul(
                            ps, z1T[:, d, tt * 128:(tt + 1) * 128], wv[:, d, :],
                            start=(d == 0), stop=(d == DT - 1),
                        )
                    dst = VP[:, tt, vc * 8 * (HD + 1):(vc + 1) * 8 * (HD + 1)]
                    dst = dst.rearrange("p (h e) -> p h e", e=HD + 1)[:, :, 0:HD]
                    src = ps.rearrange("p (h e) -> p h e", e=HD)
                    vb = vbias_sb[:, vc * 512:(vc + 1) * 512].rearrange(
                        "p (h e) -> p h e", e=HD
                    )
                    nc.vector.scalar_tensor_tensor(
                        out=dst, in0=src, scalar=0.0, in1=vb,
                        op0=ALU.bypass, op1=ALU.add,
                    )
        es_z1.close()  # z1T dead after QKV

        # ---- phase C: attention ----
        es_ao = ExitStack()
        aop = es_ao.enter_context(tc.tile_pool(name="aop", bufs=1, side="right"))
        aoT = aop.tile([128, DT, T_OWN], BF16, tag="aoT")

        with (
            tc.tile_pool(name="exps", bufs=4) as exp_pool,
            tc.tile_pool(name="nrm", bufs=2) as nrm_pool,
            tc.tile_pool(name="psCs", bufs=2, space="PSUM") as spsum,
            tc.tile_pool(name="psCa", bufs=1, space="PSUM") as avpsum,
        ):
            for h in range(H):
                ft, pr = h // 2, (h % 2) * 64
                KhT = kt_all[pr:pr + 64, ft, :]
                QhT = qt_all[pr:pr + 64, ft, :]
                av = avpsum.tile([HD + 1, T_OWN], FP32, tag="av")
                for kt in range(T_ALL // 128):
                    sp = spsum.tile([128, T_OWN], FP32, tag="sps")
                    for qc in range(2):
                        nc.tensor.matmul(
                            sp[:, qc * 512:(qc + 1) * 512],
                            KhT[:, kt * 128:(kt + 1) * 128],
                            QhT[:, qc * 512:(qc + 1) * 512],
                            start=True, stop=True,
                        )
                    ex = exp_pool.tile([128, T_OWN], BF16, tag="exp")
                    nc.scalar.activation(out=ex, in_=sp, func=AF.Exp, scale=0.125)
                    for qc in range(2):
                        nc.tensor.matmul(
                            av[:, qc * 512:(qc + 1) * 512],
                            VP[:, kt, h * (HD + 1):(h + 1) * (HD + 1)],
                            ex[:, qc * 512:(qc + 1) * 512],
                            start=(kt == 0), stop=(kt == T_ALL // 128 - 1),
                        )
                sums_sb = nrm_pool.tile([1, T_OWN], FP32, tag="sums")
                nc.vector.tensor_copy(out=sums_sb, in_=av[HD:HD + 1, :])
                rec = nrm_pool.tile([1, T_OWN], FP32, tag="rec")
                nc.vector.reciprocal_approx_fast(out=rec, in_=sums_sb)
                rec_bf = nrm_pool.tile([1, T_OWN], BF16, tag="recbf")
                nc.vector.tensor_copy(out=rec_bf, in_=rec)
                bc = avpsum.tile([128, T_OWN], FP32, tag="bc")
                for qc in range(2):
                    nc.tensor.matmul(
                        bc[:, qc * 512:(qc + 1) * 512],
                        ones_bf,
                        rec_bf[:, qc * 512:(qc + 1) * 512],
                        start=True, stop=True,
                    )
                bcs = nrm_pool.tile([128, T_OWN], FP32, tag="bcs")
                nc.vector.tensor_copy(out=bcs, in_=bc)
                nc.vector.tensor_mul(
                    out=aoT[pr:pr + 64, ft, :], in0=av[0:HD, :], in1=bcs[0:HD, :]
                )
        es_kqv.close()  # kt/qt/VP dead

        # ---- phase D: proj + residual -> x2 ----
        es_x2 = ExitStack()
        x2p = es_x2.enter_context(tc.tile_pool(name="x2p", bufs=1))
        x2_all = x2p.tile([128, T_OWN // 128, D], FP32, tag="x2")
        with (
            tc.tile_pool(name="pjw", bufs=1, side="right") as pjw_pool,
            tc.tile_pool(name="psD", bufs=2, space="PSUM") as ppsum,
        ):
            projw_sb = pjw_pool.tile([128, DT, D], BF16, tag="projw")
            nc.sync.dma_start(
                out=projw_sb, in_=wproj.ap().rearrange("(dt p) f -> p dt f", p=128)
            )
            for tt in range(T_OWN // 128):
                xo = xpool.tile([128, D], FP32, tag="xa")
                nc.sync.dma_start(out=xo, in_=x[tt * 128:(tt + 1) * 128, :])
                for oc in range(2):
                    ps = ppsum.tile([128, 512], FP32, tag="pps")
                    for d in range(DT):
                        nc.tensor.matmul(
                            ps, aoT[:, d, tt * 128:(tt + 1) * 128],
                            projw_sb[:, d, oc * 512:(oc + 1) * 512],
                            start=(d == 0), stop=False,
                        )
                    nc.tensor.matmul(
                        ps, ones_bf, bproj_sb[:, oc * 512:(oc + 1) * 512],
                        start=False, stop=True,
                    )
                    nc.vector.scalar_tensor_tensor(
                        out=x2_all[:, tt, oc * 512:(oc + 1) * 512],
                        in0=ps, scalar=0.0, in1=xo[:, oc * 512:(oc + 1) * 512],
                        op0=ALU.bypass, op1=ALU.add,
                    )
        es_ao.close()  # aoT dead

        # ---- phase E: LN2 + transpose -> z2T ----
        es_z2 = ExitStack()
        z2p = es_z2.enter_context(tc.tile_pool(name="z2p", bufs=1))
        z2T = z2p.tile([128, DT, T_OWN], BF16, tag="z2T")
        with tc.tile_pool(name="psE", bufs=2, space="PSUM") as trpsE:
            _ln_transpose(nc, statpool, znpool, trpsE,
                          lambda tt: x2_all[:, tt, :], T_OWN // 128,
                          z2T, eps_sb, ident)

        # ---- phase F: MLP ----
        with (
            tc.tile_pool(name="w1p", bufs=3) as w1_pool,
            tc.tile_pool(name="w2p", bufs=3) as w2_pool,
            tc.tile_pool(name="hp", bufs=1) as hpool,
            tc.tile_pool(name="psFh", bufs=2, space="PSUM") as hpsum,
            tc.tile_pool(name="psFo", bufs=1, space="PSUM") as opsum,
        ):
            for tc2 in range(2):
                hT = hpool.tile([128, HID // 128, 512], BF16, tag="hT")
                for jt in range(HID // 128):
                    w1f = w1_pool.tile([128, DT, 128], BF16, tag="w1f")
                    nc.sync.dma_start(
                        out=w1f, in_=w1_t[:, :, jt * 128:(jt + 1) * 128]
                    )
                    ps = hpsum.tile([128, 512], FP32, tag="hps")
                    for d in range(DT):
                        nc.tensor.matmul(
                            ps, w1f[:, d, :], z2T[:, d, tc2 * 512:(tc2 + 1) * 512],
                            start=(d == 0), stop=(d == DT - 1),
                        )
                    nc.scalar.activation(
                        out=hT[:, jt, :], in_=ps, func=AF.Gelu,
                        bias=b1_sb[:, jt:jt + 1],
                    )
                for oc in range(2):
                    pss = []
                    for i in range(4):
                        ops_t = opsum.tile([128, 512], FP32, tag=f"ops{i}")
                        pss.append(ops_t)
                    for jt in range(HID // 128):
                        w2b = w2_pool.tile([128, 512], BF16, tag="w2b")
                        nc.sync.dma_start(
                            out=w2b,
                            in_=w2_t[:, jt, oc * 512:(oc + 1) * 512],
                        )
                        for tt in range(4):
                            nc.tensor.matmul(
                                pss[tt], hT[:, jt, tt * 128:(tt + 1) * 128], w2b,
                                start=(jt == 0), stop=False,
                            )
                    for tt in range(4):
                        tglob = tc2 * 4 + tt
                        nc.tensor.matmul(
                            pss[tt], ones_bf, b2_sb[:, oc * 512:(oc + 1) * 512],
                            start=False, stop=True,
                        )
                        ys = ypool.tile([128, 512], FP32, tag="ys")
                        nc.vector.scalar_tensor_tensor(
                            out=ys, in0=pss[tt], scalar=0.0,
                            in1=x2_all[:, tglob, oc * 512:(oc + 1) * 512],
                            op0=ALU.bypass, op1=ALU.add,
                        )
                        nc.sync.dma_start(
                            out=y[tglob * 128:(tglob + 1) * 128,
                                  oc * 512:(oc + 1) * 512],
                            in_=ys,
                        )
        es_z2.close()
        es_x2.close()

    nc.compile()
    return nc


def prep_host_inputs(inputs):
    """Fold LN affine params into the adjacent matmul weights, cast to bf16,
    and build the 8 per-core input maps."""
    import ml_dtypes

    f32 = np.float32
    x = np.asarray(inputs["x"], f32)
    qkv_w = np.asarray(inputs["qkv_w"], f32)
    qkv_b = np.asarray(inputs["qkv_b"], f32)
    proj_w = np.asarray(inputs["proj_w"], f32)
    proj_b = np.asarray(inputs["proj_b"], f32)
    fc1_w = np.asarray(inputs["fc1_w"], f32)
    fc1_b = np.asarray(inputs["fc1_b"], f32)
    fc2_w = np.asarray(inputs["fc2_w"], f32)
    fc2_b = np.asarray(inputs["fc2_b"], f32)
    ln1_w = np.asarray(inputs["ln1_w"], f32)
    ln1_b = np.asarray(inputs["ln1_b"], f32)
    ln2_w = np.asarray(inputs["ln2_w"], f32)
    ln2_b = np.asarray(inputs["ln2_b"], f32)

    bf = ml_dtypes.bfloat16
    wqkv = (ln1_w[:, None] * qkv_w).astype(bf)
    bqkv = (qkv_b + ln1_b @ qkv_w).astype(f32)
    w1 = (ln2_w[:, None] * fc1_w).astype(bf)
    b1 = (fc1_b + ln2_b @ fc1_w).astype(f32)

    shared = {
        "wqkv": wqkv, "bqkv": bqkv,
        "wproj": proj_w.astype(bf), "bproj": proj_b.astype(bf),
        "w1": w1, "b1": b1,
        "w2": fc2_w.astype(bf), "b2": fc2_b.astype(bf),
    }
    in_maps = []
    for c in range(N_CORES):
        b, half = c // 2, c % 2
        own = x[b, half * 1024:(half + 1) * 1024]
        other = x[b, (1 - half) * 1024:(2 - half) * 1024]
        xc = np.concatenate([own, other], axis=0)
        in_maps.append({"x": np.ascontiguousarray(xc), **shared})
    return in_maps


# ---------------------------------------------------------------------------
# Cached PJRT runner (jit once, reuse across kernel() calls)
# ---------------------------------------------------------------------------
_CACHE = {}


def _get_runner():
    if "runner" in _CACHE:
        return _CACHE["runner"]

    from jax.experimental.shard_map import shard_map
    from jax.sharding import Mesh, PartitionSpec
    from concourse.bass2jax import (
        _bass_exec_p, install_neuronx_cc_hook, partition_id_tensor,
    )

    nc = build_nc()
    install_neuronx_cc_hook()

    partition_name = nc.partition_id_tensor.name if nc.partition_id_tensor else None
    in_names, out_names, out_avals, zero_shapes = [], [], [], []
    for alloc in nc.m.functions[0].allocations:
        if not isinstance(alloc, mybir.MemoryLocationSet):
            continue
        name = alloc.memorylocations[0].name
        if alloc.kind == "ExternalInput":
            if name != partition_name:
                in_names.append(name)
        elif alloc.kind == "ExternalOutput":
            shape = tuple(alloc.tensor_shape)
            dtype = mybir.dt.np(alloc.dtype)
            out_names.append(name)
            out_avals.append(jax.core.ShapedArray(shape, dtype))
            zero_shapes.append((shape, dtype))
    n_params = len(in_names)
    n_outs = len(out_names)
    all_in = list(in_names) + list(out_names)
    if partition_name is not None:
        all_in.append(partition_name)
    donate = tuple(range(n_params, n_params + n_outs))

    def _body(*args):
        operands = list(args)
        if partition_name is not None:
            operands.append(partition_id_tensor())
        outs = _bass_exec_p.bind(
            *operands,
            out_avals=tuple(out_avals),
            in_names=tuple(all_in),
            out_names=tuple(out_names),
            lowering_input_output_aliases=(),
            sim_require_finite=True,
            sim_require_nnan=True,
            nc=nc,
        )
        return tuple(outs)

    devices = jax.devices()[:N_CORES]
    mesh = Mesh(np.asarray(devices), ("core",))
    sharded = jax.jit(
        shard_map(
            _body, mesh=mesh,
            in_specs=(PartitionSpec("core"),) * (n_params + n_outs),
            out_specs=(PartitionSpec("core"),) * n_outs,
            check_rep=False,
        ),
        donate_argnums=donate, keep_unused=True,
    )

    def run(in_maps):
        concat_in = [
            np.concatenate([np.asarray(m[name]) for m in in_maps], axis=0)
            for name in in_names
        ]
        concat_zeros = [
            np.zeros((N_CORES * s[0], *s[1:]), dt) for (s, dt) in zero_shapes
        ]
        out_arrs = sharded(*concat_in, *concat_zeros)
        per_core = []
        for c in range(N_CORES):
            per_core.append({
                name: np.asarray(out_arrs[i]).reshape(
                    N_CORES, *out_avals[i].shape)[c]
                for i, name in enumerate(out_names)
            })
        return per_core

    _CACHE["runner"] = run
    return run


def kernel(**inputs) -> np.ndarray:
    run = _get_runner()
    in_maps = prep_host_inputs(inputs)
    results = run(in_maps)
    out = np.zeros((4, 2048, 1024), np.float32)
    for c in range(N_CORES):
        b, half = c // 2, c % 2
        out[b, half * 1024:(half + 1) * 1024] = results[c]["y"]
    return out
